# revision 23
# baseline (speedup 1.0000x reference)
"""SphericalConv (gather-based 3x3 conv + 2x nearest upsample) on 8 trn2 cores.

Strategy (data-parallel over batch, one batch image per core):
  0. The fp32 feature is pre-cast to a bf16 DRAM copy by one Pool (SWDGE)
     DRAM->DRAM casting DMA; bf16 tiles then cost half the DMA-queue time.
     The first two tiles are loaded directly as float32r (full-rate fp32
     streaming on the PE) so the PE starts before the pre-cast completes.
  1. S_k = sum_c w[c,k] * F[c,:,:] for the 9 taps, via PE matmuls with a
     block-diagonal stationary [128, 32] (two source-row halves x 9 taps in
     16-aligned column slots).  Tile loads are spread over the three DMA
     issuers (SP / Act HWDGE, Pool SWDGE) so their queues run concurrently.
  2. S rows are converted to bf16 (DVE copy out of PSUM, one copy per TWO
     row-groups) and written doubled ([row|row]) to a DRAM scratch S2X so a
     circular shift of a row is one contiguous 512-element read.  The flush
     is a single wide DMA whose DRAM access pattern leads with the 2048-row
     dim (cheap: DMA cost tracks bytes-per-leading-dim-entry).
  3. The spherical gather out[h,w] = sum_k S_k[gi(h,k), (w+d(h,k)) mod W] is
     18 indirect DMA gathers (one per (h-parity, tap)) that ACCUMULATE
     (compute_op=add) into per-parity [128, 512] tiles, so no separate
     tap-sum pass.  Offsets are computed on the host from gi/gj.
  4. Nearest-neighbor 2x upsample = strided DVE column-doubles + four bf16
     output DMAs (split per parity so the even half overlaps the odd half's
     gathers); the host upcasts to fp32.

The gi/gj maps produced by the gnomonic projection are row-structured
(gi constant along w; gj a per-row circular shift).  This is verified on the
host; arbitrary (unstructured) index maps fall back to a host computation.
"""

import sys

sys.path.insert(0, "/opt/trn_rl_repo")

import numpy as np

B, C, H, W = 8, 64, 256, 512
NCORES = 8
TAPS = 9
ROWLEN = 1024  # doubled S row (elements, bf16)
NROWS_X = 4096  # fl*2048 + p*16 + m16 (9 of 16 tap slots used; pads are zeros)
NTOT = NROWS_X * ROWLEN

_prog_cache = {}


def _split_multi_waits(nc, mybir):
    # This container's walrus rejects >1 sync wait per instruction; hoist the
    # extra waits onto standalone event-semaphore instructions just before.
    n = 0
    for blk in nc.m.functions[0].blocks:
        insts = blk.instructions
        new, changed = [], False
        for i in insts:
            si = i.sync_info
            if si is not None and len(si.on_wait) > 1:
                waits = list(si.on_wait)
                for w in waits[:-1]:
                    n += 1
                    ev = mybir.InstEventSemaphore(
                        name=f"wsplit_{n}_{i.name}",
                        engine=i.engine,
                        sync_info=mybir.SyncInfo(on_wait=[w], on_update=[]),
                    )
                    new.append(ev)
                i.sync_info = mybir.SyncInfo(
                    on_wait=[waits[-1]], on_update=list(si.on_update)
                )
                changed = True
            new.append(i)
        if changed:
            blk.instructions = new


def _build_program(split_waits=True):
    key = "nc" if split_waits else "nc_raw"
    if key in _prog_cache:
        return _prog_cache[key]

    import concourse.bass as bass
    import concourse.tile as tile
    from concourse import mybir
    from concourse.bass import AP, IndirectOffsetOnAxis

    f32r = mybir.dt.float32r
    bf16 = mybir.dt.bfloat16

    nc = bass.Bass("TRN2", target_bir_lowering=False, debug=False)
    feat = nc.dram_tensor("feat", [C, H, W], mybir.dt.float32, kind="ExternalInput")
    wbd = nc.dram_tensor("wbd", [128, 32], mybir.dt.float32, kind="ExternalInput")
    offs = nc.dram_tensor("offs", [128, 18], mybir.dt.int32, kind="ExternalInput")
    # out column-planes: out[b, h2, w] = result[h2, 2*w + b]; the host
    # interleaves the two planes (pure layout permutation)
    out = nc.dram_tensor("out", [2, 2 * H, W], bf16, kind="ExternalOutput")
    featb = nc.dram_tensor("featb", [C * H * W], bf16)  # bf16 feature copy
    s2x = nc.dram_tensor("s2x", [NTOT], bf16)  # gather scratch

    with tile.TileContext(nc) as tc:
        with (
            tc.tile_pool(name="consts", bufs=1) as consts,
            tc.tile_pool(name="ft", bufs=6) as ftp,
            tc.tile_pool(name="ftc", bufs=8) as ftcp,
            tc.tile_pool(name="ps", bufs=4, space="PSUM") as psp,
            tc.tile_pool(name="stage", bufs=2) as stp,
            tc.tile_pool(name="outp", bufs=1) as outp,
        ):
            wt16 = consts.tile([128, 32], bf16)
            nc.gpsimd.dma_start(wt16[:], wbd.ap())  # casting load


            # main loop: 32 groups x 4 row-pairs (rowA=4m+i, rowB=128+4m+i)
            st = None
            ps = None
            for m in range(32):
                # partitions 0-63 = channels for rows 4m..4m+3,
                # partitions 64-127 = channels for rows 128+4m..128+4m+3
                if m == 2:
                    # whole-feature fp32 -> bf16 cast via two cheap D2D DMAs
                    # on Pool (interleaved odd/even 512-chunks so the balancer
                    # can't merge and re-split the pattern into wide per-row
                    # transfers).  Placed after the m<2 casting loads in Pool
                    # program order.
                    for half in range(2):
                        nc.gpsimd.dma_start(
                            AP(featb, half * 512, [(1024, C * H // 2), (1, 512)]),
                            AP(feat, half * 512, [(1024, C * H // 2), (1, 512)]),
                        )
                ft = ftp.tile([128, 2048], bf16)
                if m < 2:
                    # Pool casting loads straight from fp32 feat so the PE
                    # starts before the pre-cast completes
                    src = AP(feat, 4 * m * W, [(128 * W, 2), (H * W, C), (1, 4 * W)])
                    nc.gpsimd.dma_start(ft[:], src)
                else:
                    src = AP(
                        featb, 4 * m * W, [(128 * W, 2), (H * W, C), (1, 4 * W)]
                    )
                    eng = (nc.sync, nc.scalar, nc.gpsimd, nc.sync, nc.scalar)[
                        m % 5
                    ]
                    eng.dma_start(ft[:], src)
                rhs = [ft[:, 512 * i : 512 * (i + 1)] for i in range(4)]
                wt = wt16

                if m % 2 == 0:
                    ps = psp.tile([128, 1024], mybir.dt.float32)
                pso = (m % 2) * 512
                for i in range(4):
                    nc.tensor.matmul(
                        ps[32 * i : 32 * i + 32, pso : pso + 512],
                        lhsT=wt[:],
                        rhs=rhs[i],
                        start=True,
                        stop=True,
                        # base_partition auto-derive caps at 64; pass explicitly
                        tile_position=(0, 32 * i),
                    )

                fl = m // 16
                mm = m % 16
                if mm == 0:
                    st = stp.tile([128, 16 * 512], bf16)
                if m % 2 == 1:
                    nc.vector.tensor_copy(
                        st[:, (mm - 1) * 512 : (mm + 1) * 512], ps[:]
                    )

                if mm == 15:
                    for dbl in range(2):
                        dst = AP(
                            s2x,
                            fl * 2048 * ROWLEN + dbl * 512,
                            [(ROWLEN, 2048), (1, 512)],
                        )
                        (nc.scalar if dbl == 0 else nc.sync).dma_start(dst, st[:])

            offs_t = consts.tile([128, 18], mybir.dt.int32)
            nc.sync.dma_start(offs_t[:], offs.ap())

            # gather-accumulate: 9 taps, both parities per instruction;
            # offs column s = 2*k + hd, out slot o[p, hd, :], h = 2p + hd
            o = outp.tile([128, 2, 512], bf16)
            for k in range(9):
                nc.gpsimd.indirect_dma_start(
                    out=o[:],
                    out_offset=None,
                    in_=AP(s2x, 0, [(512, NTOT // 512), (1, 512)]),
                    in_offset=IndirectOffsetOnAxis(
                        ap=offs_t[:, 2 * k : 2 * k + 2], axis=1
                    ),
                    compute_op=(
                        mybir.AluOpType.bypass if k == 0 else mybir.AluOpType.add
                    ),
                )

            # 2x upsample: each conv row is written 4x (2 row-copies x 2
            # column-planes); out row = 4p + 2*hd + a within plane b
            engs = [
                nc.gpsimd, nc.sync, nc.scalar, nc.gpsimd,
                nc.sync, nc.scalar, nc.gpsimd, nc.gpsimd,
            ]
            n = 0
            for hd in range(2):
                for a in range(2):
                    for b in range(2):
                        dst = AP(
                            out,
                            b * (2 * H * W) + (2 * hd + a) * W,
                            [(4 * W, 128), (1, W)],
                        )
                        engs[n].dma_start(dst, o[:, hd, :])
                        n += 1

    if split_waits:
        _split_multi_waits(nc, mybir)
    _prog_cache[key] = nc
    return nc


def _structured(gi, gj):
    if not all(np.array_equal(gi[:, :, k], np.broadcast_to(gi[:, :1, k], (H, W))) for k in range(TAPS)):
        return False
    d = (gj - np.arange(W, dtype=np.int64)[None, :, None]) % W
    return all(np.array_equal(d[:, :, k], np.broadcast_to(d[:, :1, k], (H, W))) for k in range(TAPS))


def _host_fallback(feature, weight, gi, gj):
    # correct-but-slow path for arbitrary (non roll-structured) index maps
    wflat = weight.reshape(1, C, TAPS).astype(np.float32)
    outc = np.zeros((B, H, W), np.float32)
    for k in range(TAPS):
        xk = feature[:, :, gi[:, :, k], gj[:, :, k]]
        outc += np.einsum("bchw,c->bhw", xk, wflat[0, :, k])
    up = np.repeat(np.repeat(outc, 2, axis=1), 2, axis=2)
    return up[:, None].astype(np.float32)


def _make_device_inputs(weight, gi, gj):
    # block-diag stationary [128, 32]: wt[64*t9 + c, 16*t9 + k] = w[c,k]
    w9 = np.asarray(weight, np.float32).reshape(C, TAPS)
    wbd = np.zeros((128, 32), np.float32)
    for t9 in range(2):
        wbd[64 * t9 : 64 * t9 + 64, 16 * t9 : 16 * t9 + 9] = w9

    r = gi[:, 0, :].astype(np.int64)  # [H, 9]
    d = gj[:, 0, :].astype(np.int64) % W  # shift per (h, k)

    # S2X row id for source row r, tap k: p*16 + m16 (+ fl*2048),
    # p = 32*i4 + 16*t9r + k
    t9r = r // 128
    rr = r % 128
    i4 = rr % 4
    mm = rr // 4
    fl = mm // 16
    m16 = mm % 16
    row_id = fl * 2048 + ((i4 * 2 + t9r) * 16 + np.arange(TAPS)[None, :]) * 16 + m16
    off_hk = row_id * ROWLEN + d  # [H, 9]

    offs = np.zeros((128, 18), np.int32)
    for hd in range(2):
        for k in range(TAPS):
            offs[:, 2 * k + hd] = off_hk[2 * np.arange(128) + hd, k]
    return wbd, offs


def _run_device(feature, wbd, offs, trace=False, trace_kwargs=None):
    from concourse.bass_utils import run_bass_kernel_spmd

    nc = _build_program()
    in_maps = [
        {"feat": np.ascontiguousarray(feature[b]), "wbd": wbd, "offs": offs}
        for b in range(B)
    ]
    kw = {}
    if trace:
        kw["trace"] = True
        if trace_kwargs:
            kw.update(trace_kwargs)
    return run_bass_kernel_spmd(nc, in_maps, list(range(NCORES)), **kw)


def kernel(feature, weight, gi, gj):
    feature = np.asarray(feature, dtype=np.float32)
    weight = np.asarray(weight, dtype=np.float32)
    gi = np.asarray(gi)
    gj = np.asarray(gj)

    if not _structured(gi, gj):
        return _host_fallback(feature, weight, gi, gj)

    wbd, offs = _make_device_inputs(weight, gi, gj)
    res = _run_device(feature, wbd, offs)
    out = np.empty((B, 1, 2 * H, 2 * W), np.float32)
    for b in range(B):
        planes = np.asarray(res.results[b]["out"]).astype(np.float32)
        out[b, 0, :, 0::2] = planes[0]
        out[b, 0, :, 1::2] = planes[1]
    return out
